# revision 19
# baseline (speedup 1.0000x reference)
"""DeepSeekV3-style MoE layer (1 MoE block) on 8 Trainium2 NeuronCores.

Sharding: expert-parallel. Each core owns 4 of the 32 routed experts and a
64-wide shard of the shared expert's intermediate dim. The router is
replicated (router weight columns are permuted per-core so the local experts
always sit in columns 0..3 — top-k and sigmoid are permutation invariant).
Partial outputs are combined with one on-device ReduceScatter; the host
reassembles the 8 output shards.

Host-side prep (layout/dtype only, in prep_inputs): x is cast to bf16 plus a
bf16 residual (router accuracy), transposed to feature-major and laid out
partition-major per chunk so every device DMA is contiguous 8 KiB rows; all
weights are pre-cast to bf16 (router as a split-bf16 pair w1+w2 ~= Wr fp32)
and laid out partition-major. The device kernel is pure matmul + top-k +
elementwise — no on-device transposes, casts, or strided DMA.

Per-core device pipeline (feature-major activations):
  - router runs in split-bf16 with ~fp32 accuracy:
    logits = w1.x1 + w1.x2 + w2.x1 accumulated in one fp32 PSUM group
  - top-8 selection on logits via iterative max extraction (sigmoid is
    monotonic so logit order == affinity order), normalized sigmoid weights
  - bf16 gate/up matmuls -> silu(g+bg) * (u+bu) * token_weight -> bf16 hge
  - down-projection with hge as the stationary operand so the PSUM output is
    token-major [128 tokens x H], accumulating all 4 experts + shared expert
    + bias trick ([w_e rows; ones] @ [bd_e rows; bd_shared]) in one group.
"""

import sys

sys.path.insert(0, "/opt/trn_rl_repo")

import numpy as np

import concourse.bacc as bacc
import concourse.bass as bass
import concourse.mybir as mybir
import concourse.tile as tile
from concourse.masks import make_identity

F32 = mybir.dt.float32
BF16 = mybir.dt.bfloat16
AF = mybir.ActivationFunctionType
ALU = mybir.AluOpType

H, I, E, TOPK = 1024, 512, 32, 8
B, S = 4, 1024
T = B * S
NCORES = 8
E_LOC = E // NCORES          # 4 routed experts per core
I_SH = I // NCORES           # 64-wide shared-expert shard per core
P = 128
TC = 512                     # token chunk
NCH = T // TC                # 8 chunks
NH = H // P                  # 8 hidden k-tiles
NI = I // P                  # 4 intermediate tiles
NJ = TC // P                 # 4 token tiles per chunk
T_SHARD = T // NCORES        # 512 rows per core after ReduceScatter
NEG = -1.0e30


def build_nc():
    nc = bacc.Bacc(None, target_bir_lowering=False, num_devices=NCORES)

    # all DRAM layouts are partition-major with contiguous per-partition rows
    xtb_d = nc.declare_dram_parameter("xtb", [NCH, P, NH * TC], BF16, isOutput=False)
    xtr_d = nc.declare_dram_parameter("xtr", [NCH, P, NH * TC], BF16, isOutput=False)
    wr1_d = nc.declare_dram_parameter("wr1", [P, NH * E], BF16, isOutput=False)
    wr2_d = nc.declare_dram_parameter("wr2", [P, NH * E], BF16, isOutput=False)
    br_d = nc.declare_dram_parameter("br", [E, 1], F32, isOutput=False)
    wg_d = nc.declare_dram_parameter("wg", [E_LOC, P, NH * I], BF16, isOutput=False)
    wu_d = nc.declare_dram_parameter("wu", [E_LOC, P, NH * I], BF16, isOutput=False)
    wd_d = nc.declare_dram_parameter("wd", [E_LOC, P, NI * H], BF16, isOutput=False)
    bg_d = nc.declare_dram_parameter("bg", [P, E_LOC * NI], F32, isOutput=False)
    bu_d = nc.declare_dram_parameter("bu", [P, E_LOC * NI], F32, isOutput=False)
    bias5_d = nc.declare_dram_parameter("bias5", [E_LOC + 1, H], BF16, isOutput=False)
    wgu_d = nc.declare_dram_parameter("wgu", [P, NH * 2 * I_SH], BF16, isOutput=False)
    wds_d = nc.declare_dram_parameter("wds", [I_SH, H], BF16, isOutput=False)
    bgs_d = nc.declare_dram_parameter("bgs", [I_SH, 1], F32, isOutput=False)
    bus_d = nc.declare_dram_parameter("bus", [I_SH, 1], F32, isOutput=False)
    sel_d = nc.declare_dram_parameter("sel", [E_LOC, E_LOC * P], BF16, isOutput=False)
    y_d = nc.declare_dram_parameter("y", [T_SHARD, H], F32, isOutput=True)

    cc_ins = [nc.dram_tensor(f"cc_in{ch}", [TC, H], F32) for ch in range(NCH)]
    cc_outs = [nc.dram_tensor(f"cc_out{ch}", [TC // NCORES, H], F32)
               for ch in range(NCH)]

    with tile.TileContext(nc) as tc:
        with (
            tc.tile_pool(name="wres", bufs=1) as wres,
            tc.tile_pool(name="xtcp", bufs=3) as xtcp,
            tc.tile_pool(name="xtrp", bufs=2) as xtrp,
            tc.tile_pool(name="hgep", bufs=1) as hgep,
            tc.tile_pool(name="actp", bufs=2) as actp,
            tc.tile_pool(name="outp", bufs=2) as outp,
            tc.tile_pool(name="rtp", bufs=2) as rtp,
            tc.tile_pool(name="ps_tr", bufs=1, space="PSUM") as ps_tr,
            tc.tile_pool(name="ps_r", bufs=1, space="PSUM") as ps_r,
            tc.tile_pool(name="ps_g", bufs=2, space="PSUM") as ps_g,
            tc.tile_pool(name="ps_u", bufs=2, space="PSUM") as ps_u,
            tc.tile_pool(name="ps_d", bufs=1, space="PSUM") as ps_d,
        ):
            # ---------- constants / small weights ----------
            ident = wres.tile([P, P], F32, tag="ident")
            make_identity(nc, ident[:])

            # router weights first — the PE's first work is router(0)
            wr1_sb = wres.tile([P, NH * E], BF16, tag="wr1")
            nc.sync.dma_start(wr1_sb[:], wr1_d[:])
            wr2_sb = wres.tile([P, NH * E], BF16, tag="wr2")
            nc.sync.dma_start(wr2_sb[:], wr2_d[:])

            # per-chunk feature-major x + residual (bf16), contiguous DMAs.
            # Loaded as per-h-slice DMAs so consumers of early h slices can
            # start before the whole chunk has landed.
            def load_xt(ch):
                xtc = xtcp.tile([P, NH * TC], BF16, tag="xtc", name="xtc")
                xtr = xtrp.tile([P, NH * TC], BF16, tag="xtr", name="xtr")
                for h in range(NH):
                    s = slice(h * TC, (h + 1) * TC)
                    nc.sync.dma_start(xtc[:, s], xtb_d[ch][:, s])
                    nc.sync.dma_start(xtr[:, s], xtr_d[ch][:, s])
                return xtc, xtr

            x_chunks = {0: load_xt(0), 1: load_xt(1)}

            # selector constant for per-expert weight-row broadcast
            sel_bf = wres.tile([E_LOC, E_LOC * P], BF16, tag="sel")
            nc.sync.dma_start(sel_bf[:], sel_d[:])

            # biases
            br_sb = wres.tile([E, 1], F32, tag="br")
            nc.sync.dma_start(br_sb[:], br_d[:])
            bg_sb = wres.tile([P, E_LOC * NI], F32, tag="bg")
            nc.sync.dma_start(bg_sb[:], bg_d[:])
            bu_sb = wres.tile([P, E_LOC * NI], F32, tag="bu")
            nc.sync.dma_start(bu_sb[:], bu_d[:])
            bgs_sb = wres.tile([I_SH, 1], F32, tag="bgs")
            nc.sync.dma_start(bgs_sb[:], bgs_d[:])
            bus_sb = wres.tile([I_SH, 1], F32, tag="bus")
            nc.sync.dma_start(bus_sb[:], bus_d[:])
            bias5_sb = wres.tile([E_LOC + 1, H], BF16, tag="bias5")
            nc.sync.dma_start(bias5_sb[:], bias5_d[:])

            # routing weights, feature-major: rows 0..3 local expert w, row 4 ones
            we_sb = wres.tile([E_LOC + 1, T], BF16, tag="we")
            nc.vector.memset(we_sb[:], 1.0)

            # ---------- resident expert weights (direct bf16 loads) ----------
            wg_bf = {}
            wu_bf = {}
            wd_bf = {}
            for e in range(E_LOC):
                for name, dram, store in (("wg", wg_d, wg_bf), ("wu", wu_d, wu_bf)):
                    res = wres.tile([P, NH * I], BF16, tag=f"{name}{e}")
                    nc.scalar.dma_start(res[:], dram[e])
                    store[e] = res
                res = wres.tile([P, NI * H], BF16, tag=f"wd{e}")
                nc.scalar.dma_start(res[:], wd_d[e])
                wd_bf[e] = res
            wgu_sb = wres.tile([P, NH * 2 * I_SH], BF16, tag="wgu")
            nc.scalar.dma_start(wgu_sb[:], wgu_d[:])
            wds_sb = wres.tile([I_SH, H], BF16, tag="wds")
            nc.scalar.dma_start(wds_sb[:], wds_d[:])

            def router(ch, xtc, xtr):
                t0 = ch * TC
                pr = ps_r.tile([E, TC], F32, tag="r", name="pr")
                for h in range(NH):
                    nc.tensor.matmul(pr[:], wr1_sb[:, h * E:(h + 1) * E],
                                     xtc[:, h * TC:(h + 1) * TC],
                                     start=(h == 0), stop=False)
                    nc.tensor.matmul(pr[:], wr1_sb[:, h * E:(h + 1) * E],
                                     xtr[:, h * TC:(h + 1) * TC],
                                     start=False, stop=False)
                    nc.tensor.matmul(pr[:], wr2_sb[:, h * E:(h + 1) * E],
                                     xtc[:, h * TC:(h + 1) * TC],
                                     start=False, stop=(h == NH - 1))
                logits_fm = rtp.tile([E, TC], F32, tag="logits_fm", bufs=1)
                nc.scalar.activation(logits_fm[:], pr[:], AF.Identity,
                                     bias=br_sb[:, 0:1])
                # transpose to token-major [128, 4, 32]
                logits_tm = rtp.tile([P, NJ, E], F32, tag="logits_tm")
                for j in range(NJ):
                    pt = ps_tr.tile([P, E], F32, tag="tr", name="ptl")
                    nc.tensor.transpose(pt[:], logits_fm[:, j * P:(j + 1) * P],
                                        ident[0:E, 0:E])
                    nc.vector.tensor_copy(logits_tm[:, j, :], pt[:])
                # top-8 threshold by iterative max extraction
                cur = rtp.tile([P, NJ, E], F32, tag="cur")
                nc.vector.tensor_copy(cur[:], logits_tm[:])
                mx = rtp.tile([P, NJ], F32, tag="mx")
                mask = rtp.tile([P, NJ, E], F32, tag="mask", bufs=1)
                for k in range(TOPK):
                    nc.vector.tensor_reduce(mx[:], cur[:], mybir.AxisListType.X,
                                            ALU.max)
                    if k < TOPK - 1:
                        mxb = mx[:].rearrange("p (f o) -> p f o", o=1).broadcast_to(
                            [P, NJ, E])
                        nc.vector.tensor_tensor(mask[:], cur[:], mxb, ALU.is_ge)
                        nc.vector.scalar_tensor_tensor(cur[:], mask[:], NEG, cur[:],
                                                       ALU.mult, ALU.add)
                # mask8 / normalized sigmoid weights
                aff = rtp.tile([P, NJ, E], F32, tag="aff")
                nc.scalar.activation(aff[:], logits_tm[:], AF.Sigmoid)
                thrb = mx[:].rearrange("p (f o) -> p f o", o=1).broadcast_to(
                    [P, NJ, E])
                nc.vector.tensor_tensor(mask[:], logits_tm[:], thrb, ALU.is_ge)
                nc.vector.tensor_tensor(aff[:], aff[:], mask[:], ALU.mult)
                den = rtp.tile([P, NJ], F32, tag="den")
                nc.vector.tensor_reduce(den[:], aff[:], mybir.AxisListType.X, ALU.add)
                rec = rtp.tile([P, NJ], F32, tag="rec")
                nc.vector.reciprocal(rec[:], den[:])
                recb = rec[:].rearrange("p (f o) -> p f o", o=1).broadcast_to(
                    [P, NJ, E])
                w_tm = rtp.tile([P, NJ, E], F32, tag="w_tm")
                nc.vector.tensor_tensor(w_tm[:], aff[:], recb, ALU.mult)
                # local expert weights, feature-major -> we_sb rows 0..3 (bf16)
                for j in range(NJ):
                    pt = ps_tr.tile([E_LOC, P], F32, tag="tr", name="ptw")
                    nc.tensor.transpose(pt[:], w_tm[:, j, 0:E_LOC], ident[:])
                    nc.vector.tensor_copy(
                        we_sb[0:E_LOC, t0 + j * P:t0 + (j + 1) * P], pt[:])

            def experts(ch, xtc):
                t0 = ch * TC
                # gate/up -> hge (bf16)
                hge = {}
                for e in range(E_LOC):
                    # broadcast token-weight row -> [128, TC] via selector matmul
                    pw = ps_r.tile([P, TC], F32, tag="r", name="pw")
                    nc.tensor.matmul(pw[:], sel_bf[:, e * P:(e + 1) * P],
                                     we_sb[0:E_LOC, t0:t0 + TC],
                                     start=True, stop=True)
                    w_bc = actp.tile([P, TC], BF16, tag="w_bc", bufs=1)
                    nc.vector.tensor_copy(w_bc[:], pw[:])
                    for i in range(NI):
                        pg = ps_g.tile([P, TC], F32, tag="g")
                        pu = ps_u.tile([P, TC], F32, tag="u")
                        for h in range(NH):
                            nc.tensor.matmul(pg[:],
                                             wg_bf[e][:, h * I + i * P:h * I + (i + 1) * P],
                                             xtc[:, h * TC:(h + 1) * TC],
                                             start=(h == 0),
                                             stop=(h == NH - 1))
                        for h in range(NH):
                            nc.tensor.matmul(pu[:],
                                             wu_bf[e][:, h * I + i * P:h * I + (i + 1) * P],
                                             xtc[:, h * TC:(h + 1) * TC],
                                             start=(h == 0),
                                             stop=(h == NH - 1))
                        g_act = actp.tile([P, TC], F32, tag="g_act")
                        nc.scalar.activation(g_act[:], pg[:], AF.Silu,
                                             bias=bg_sb[:, e * NI + i:e * NI + i + 1])
                        u_w = actp.tile([P, TC], F32, tag="u_w")
                        nc.vector.scalar_tensor_tensor(
                            u_w[:], pu[:], bu_sb[:, e * NI + i:e * NI + i + 1],
                            w_bc[:], ALU.add, ALU.mult)
                        ht = hgep.tile([P, TC], BF16, tag=f"hge{e}_{i}", name="ht")
                        nc.vector.tensor_tensor(ht[:], g_act[:], u_w[:], ALU.mult)
                        hge[(e, i)] = ht

                # shared expert shard -> hge_s (bf16, 64 partitions).
                # gate and up are packed into one [H, 128] stationary block
                # (rows 0:64 gate, 64:128 up) so the PE array runs full-width.
                psgu = ps_g.tile([2 * I_SH, TC], F32, tag="g", name="psgu")
                for h in range(NH):
                    nc.tensor.matmul(
                        psgu[:], wgu_sb[:, h * 2 * I_SH:(h + 1) * 2 * I_SH],
                        xtc[:, h * TC:(h + 1) * TC],
                        start=(h == 0), stop=(h == NH - 1))
                gs = actp.tile([I_SH, TC], F32, tag="gs", bufs=1)
                nc.scalar.activation(gs[:], psgu[0:I_SH, :], AF.Silu,
                                     bias=bgs_sb[:, 0:1])
                hs = hgep.tile([I_SH, TC], BF16, tag="hge_s")
                nc.vector.scalar_tensor_tensor(hs[:], psgu[I_SH:2 * I_SH, :],
                                               bus_sb[:, 0:1], gs[:],
                                               ALU.add, ALU.mult)

                # down projection, token-major output
                for j in range(NJ):
                    ts = t0 + j * P
                    out_sb = outp.tile([P, H], F32, tag="out")
                    for half in range(2):
                        hs0 = half * (H // 2)
                        pd = ps_d.tile([P, H // 2], F32, tag=f"d{half}",
                                       name=f"pd{half}")
                        m = 0
                        for e in range(E_LOC):
                            for i in range(NI):
                                nc.tensor.matmul(
                                    pd[:],
                                    hge[(e, i)][:, j * P:(j + 1) * P],
                                    wd_bf[e][:, i * H + hs0:i * H + hs0 + H // 2],
                                    start=(m == 0), stop=False)
                                m += 1
                        nc.tensor.matmul(pd[:],
                                         hs[:, j * P:(j + 1) * P],
                                         wds_sb[:, hs0:hs0 + H // 2],
                                         start=False, stop=False)
                        nc.tensor.matmul(pd[:],
                                         we_sb[:, ts:ts + P],
                                         bias5_sb[:, hs0:hs0 + H // 2],
                                         start=False, stop=True)
                        nc.vector.tensor_copy(out_sb[:, hs0:hs0 + H // 2], pd[:])
                    ch_i, off = divmod(ts, TC)
                    nc.gpsimd.dma_start(cc_ins[ch_i][off:off + P, :], out_sb[:])

            # ---------- main loop ----------
            def reduce_chunk(ch):
                nc.gpsimd.collective_compute(
                    "ReduceScatter",
                    ALU.add,
                    ins=[cc_ins[ch][:]],
                    outs=[cc_outs[ch][:]],
                    replica_groups=[list(range(NCORES))],
                )
                rows = TC // NCORES
                nc.scalar.dma_start(y_d[ch * rows:(ch + 1) * rows, :],
                                    cc_outs[ch][:])

            # routers 0-3 run up front: ~24 us of PE work that overlaps the
            # expert-weight DMA stream, instead of the PE stalling on wg0/wu0
            router(0, *x_chunks[0])
            router(1, *x_chunks[1])
            for ch in range(NCH):
                if ch + 2 < NCH:
                    x_chunks[ch + 2] = load_xt(ch + 2)
                xtc, _ = x_chunks.pop(ch)
                experts(ch, xtc)
                if ch + 2 < NCH:
                    router(ch + 2, *x_chunks[ch + 2])
                reduce_chunk(ch)

    nc.finalize()
    return nc


def _pm(a, p=P):
    """[..., n*p, cols] -> partition-major [..., p, n*cols] with contiguous
    per-partition rows."""
    *lead, rows, cols = a.shape
    n = rows // p
    return np.ascontiguousarray(
        a.reshape(*lead, n, p, cols).swapaxes(-3, -2).reshape(*lead, p, n * cols))


def prep_inputs(inputs):
    """Split/replicate full inputs into 8 per-core input maps (layout + dtype
    prep only — bf16 casts, transposes, partition-major relayouts)."""
    import ml_dtypes

    bf16 = ml_dtypes.bfloat16
    hs = np.ascontiguousarray(np.asarray(inputs["hidden_states"], dtype=np.float32))
    x = hs.reshape(T, H)
    x_bf = x.astype(bf16)
    x_r = (x - x_bf.astype(np.float32)).astype(bf16)

    def xlayout(xt):
        # [H, T] -> [NCH, P, NH*TC]
        return np.ascontiguousarray(
            xt.reshape(NH, P, NCH, TC).transpose(2, 1, 0, 3).reshape(NCH, P, NH * TC))

    xtb = xlayout(np.ascontiguousarray(x_bf.T))
    xtr = xlayout(np.ascontiguousarray(x_r.T))
    Wr = np.asarray(inputs["Wr"], np.float32)
    br = np.asarray(inputs["br"], np.float32)
    Wg = np.asarray(inputs["Wg"], np.float32)
    bg = np.asarray(inputs["bg"], np.float32)
    Wu = np.asarray(inputs["Wu"], np.float32)
    bu = np.asarray(inputs["bu"], np.float32)
    Wd = np.asarray(inputs["Wd"], np.float32)
    bd = np.asarray(inputs["bd"], np.float32)
    Wg_s = np.asarray(inputs["Wg_s"], np.float32)
    bg_s = np.asarray(inputs["bg_s"], np.float32)
    Wu_s = np.asarray(inputs["Wu_s"], np.float32)
    bu_s = np.asarray(inputs["bu_s"], np.float32)
    Wd_s = np.asarray(inputs["Wd_s"], np.float32)
    bd_s = np.asarray(inputs["bd_s"], np.float32)

    sel = np.kron(np.eye(E_LOC, dtype=np.float32),
                  np.ones((1, P), dtype=np.float32)).astype(bf16)

    in_maps = []
    for c in range(NCORES):
        loc = list(range(c * E_LOC, (c + 1) * E_LOC))
        rest = [e for e in range(E) if e not in loc]
        perm = loc + rest
        sh = slice(c * I_SH, (c + 1) * I_SH)
        wr_f = np.ascontiguousarray(Wr[:, perm])
        wr1 = wr_f.astype(bf16)
        wr2 = (wr_f - wr1.astype(np.float32)).astype(bf16)
        bias5 = np.concatenate(
            [bd[loc], (bd_s if c == 0 else np.zeros_like(bd_s))[None, :]],
            axis=0).astype(bf16)
        in_maps.append({
            "xtb": xtb,
            "xtr": xtr,
            "wr1": _pm(wr1),
            "wr2": _pm(wr2),
            "br": np.ascontiguousarray(br[perm])[:, None],
            "wg": _pm(Wg[loc].astype(bf16)),
            "wu": _pm(Wu[loc].astype(bf16)),
            "wd": _pm(Wd[loc].astype(bf16)),
            "bg": np.ascontiguousarray(
                bg[loc].reshape(E_LOC, NI, P).transpose(2, 0, 1).reshape(P, E_LOC * NI)),
            "bu": np.ascontiguousarray(
                bu[loc].reshape(E_LOC, NI, P).transpose(2, 0, 1).reshape(P, E_LOC * NI)),
            "bias5": np.ascontiguousarray(bias5),
            "wgu": _pm(np.concatenate([Wg_s[:, sh], Wu_s[:, sh]],
                                       axis=1).astype(bf16)),
            "wds": np.ascontiguousarray(Wd_s[sh, :].astype(bf16)),
            "bgs": np.ascontiguousarray(bg_s[sh])[:, None],
            "bus": np.ascontiguousarray(bu_s[sh])[:, None],
            "sel": sel,
        })
    return in_maps


def assemble_output(results):
    """Reassemble [T, H]: 8 per-chunk ReduceScatters give core c the rows
    [ch*512 + c*64 : ch*512 + (c+1)*64] in y rows [ch*64:(ch+1)*64]."""
    ys = np.stack([results[c]["y"] for c in range(NCORES)])      # [8, 512, H]
    rows = TC // NCORES                                           # 64
    return np.ascontiguousarray(
        ys.reshape(NCORES, NCH, rows, H).transpose(1, 0, 2, 3).reshape(T, H))


_CACHE = {}


def get_runner():
    """Build + jit once; returns run(in_maps) -> list of per-core output dicts."""
    if "run" in _CACHE:
        return _CACHE["run"]
    import jax
    from jax.sharding import Mesh, PartitionSpec
    from jax.experimental.shard_map import shard_map
    from concourse import bass2jax

    nc = build_nc()
    bass2jax.install_neuronx_cc_hook()

    in_names = []
    out_names = []
    out_avals = []
    partition_name = nc.partition_id_tensor.name if nc.partition_id_tensor else None
    for alloc in nc.m.functions[0].allocations:
        if not isinstance(alloc, mybir.MemoryLocationSet):
            continue
        name = alloc.memorylocations[0].name
        if alloc.kind == "ExternalInput":
            if name != partition_name:
                in_names.append(name)
        elif alloc.kind == "ExternalOutput":
            out_names.append(name)
            out_avals.append(
                jax.core.ShapedArray(tuple(alloc.tensor_shape),
                                     mybir.dt.np(alloc.dtype)))
    n_params = len(in_names)
    n_outs = len(out_names)
    all_names = in_names + out_names + ([partition_name] if partition_name else [])
    donate = tuple(range(n_params, n_params + n_outs))

    def _body(*args):
        operands = list(args)
        if partition_name is not None:
            operands.append(bass2jax.partition_id_tensor())
        return tuple(bass2jax._bass_exec_p.bind(
            *operands,
            out_avals=tuple(out_avals),
            in_names=tuple(all_names),
            out_names=tuple(out_names),
            lowering_input_output_aliases=(),
            sim_require_finite=True,
            sim_require_nnan=True,
            nc=nc,
        ))

    devices = jax.devices()[:NCORES]
    mesh = Mesh(np.asarray(devices), ("core",))
    in_specs = (PartitionSpec("core"),) * (n_params + n_outs)
    out_specs = (PartitionSpec("core"),) * n_outs
    sharded = jax.jit(
        shard_map(_body, mesh=mesh, in_specs=in_specs, out_specs=out_specs,
                  check_rep=False),
        donate_argnums=donate, keep_unused=True)

    def run(in_maps, dev_inputs=None):
        if dev_inputs is None:
            dev_inputs = [
                np.concatenate([np.asarray(in_maps[c][n]) for c in range(NCORES)],
                               axis=0)
                for n in in_names
            ]
        zeros = [np.zeros((NCORES * a.shape[0], *a.shape[1:]), a.dtype)
                 for a in out_avals]
        outs = sharded(*dev_inputs, *zeros)
        return [
            {name: np.asarray(outs[i]).reshape(NCORES, *out_avals[i].shape)[c]
             for i, name in enumerate(out_names)}
            for c in range(NCORES)
        ]

    _CACHE["run"] = run
    _CACHE["meta"] = (in_names, out_names, out_avals, sharded, mesh)
    return run


def kernel(**inputs) -> np.ndarray:
    run = get_runner()
    in_maps = prep_inputs(inputs)
    results = run(in_maps)
    return assemble_output(results).reshape(B, S, H).astype(np.float32)


# revision 22
# speedup vs baseline: 1.0454x; 1.0454x over previous
"""DeepSeekV3-style MoE layer (1 MoE block) on 8 Trainium2 NeuronCores.

Sharding: expert-parallel. Each core owns 4 of the 32 routed experts and a
64-wide shard of the shared expert's intermediate dim. The router is
replicated (router weight columns are permuted per-core so the local experts
always sit in columns 0..3 — top-k and sigmoid are permutation invariant).
Partial outputs are combined with one on-device ReduceScatter; the host
reassembles the 8 output shards.

Host-side prep (layout/dtype only, in prep_inputs): x is cast to bf16 plus a
bf16 residual (router accuracy), transposed to feature-major and laid out
partition-major per chunk so every device DMA is contiguous 8 KiB rows; all
weights are pre-cast to bf16 (router as a split-bf16 pair w1+w2 ~= Wr fp32)
and laid out partition-major. The device kernel is pure matmul + top-k +
elementwise — no on-device transposes, casts, or strided DMA.

Per-core device pipeline (feature-major activations):
  - router runs in split-bf16 with ~fp32 accuracy:
    logits = w1.x1 + w1.x2 + w2.x1 accumulated in one fp32 PSUM group
  - top-8 selection on logits via iterative max extraction (sigmoid is
    monotonic so logit order == affinity order), normalized sigmoid weights
  - bf16 gate/up matmuls -> silu(g+bg) * (u+bu) * token_weight -> bf16 hge
  - down-projection with hge as the stationary operand so the PSUM output is
    token-major [128 tokens x H], accumulating all 4 experts + shared expert
    + bias trick ([w_e rows; ones] @ [bd_e rows; bd_shared]) in one group.
"""

import sys

sys.path.insert(0, "/opt/trn_rl_repo")

import numpy as np

import concourse.bacc as bacc
import concourse.bass as bass
import concourse.mybir as mybir
import concourse.tile as tile
from concourse.masks import make_identity

F32 = mybir.dt.float32
BF16 = mybir.dt.bfloat16
AF = mybir.ActivationFunctionType
ALU = mybir.AluOpType

H, I, E, TOPK = 1024, 512, 32, 8
B, S = 4, 1024
T = B * S
NCORES = 8
E_LOC = E // NCORES          # 4 routed experts per core
I_SH = I // NCORES           # 64-wide shared-expert shard per core
P = 128
TC = 512                     # token chunk
NCH = T // TC                # 8 chunks
NH = H // P                  # 8 hidden k-tiles
NI = I // P                  # 4 intermediate tiles
NJ = TC // P                 # 4 token tiles per chunk
T_SHARD = T // NCORES        # 512 rows per core after ReduceScatter
NEG = -1.0e30


def build_nc():
    nc = bacc.Bacc(None, target_bir_lowering=False, num_devices=NCORES)

    # all DRAM layouts are partition-major with contiguous per-partition rows
    xtb_d = nc.declare_dram_parameter("xtb", [NCH, P, NH * TC], BF16, isOutput=False)
    xtr_d = nc.declare_dram_parameter("xtr", [NCH, P, NH * TC], BF16, isOutput=False)
    wr1_d = nc.declare_dram_parameter("wr1", [P, NH * E], BF16, isOutput=False)
    wr2_d = nc.declare_dram_parameter("wr2", [P, NH * E], BF16, isOutput=False)
    br_d = nc.declare_dram_parameter("br", [E, 1], F32, isOutput=False)
    wg_d = nc.declare_dram_parameter("wg", [E_LOC, P, NH * I], BF16, isOutput=False)
    wu_d = nc.declare_dram_parameter("wu", [E_LOC, P, NH * I], BF16, isOutput=False)
    wd_d = nc.declare_dram_parameter("wd", [E_LOC, P, NI * H], BF16, isOutput=False)
    bg_d = nc.declare_dram_parameter("bg", [P, E_LOC * NI], F32, isOutput=False)
    bu_d = nc.declare_dram_parameter("bu", [P, E_LOC * NI], F32, isOutput=False)

    wgu_d = nc.declare_dram_parameter("wgu", [P, NH * 2 * I_SH], BF16, isOutput=False)
    wdsb5_d = nc.declare_dram_parameter("wdsb5", [E_LOC + 1 + I_SH, H], BF16,
                                        isOutput=False)
    bgs_d = nc.declare_dram_parameter("bgs", [I_SH, 1], F32, isOutput=False)
    bus_d = nc.declare_dram_parameter("bus", [I_SH, 1], F32, isOutput=False)
    sel_d = nc.declare_dram_parameter("sel", [E_LOC, E_LOC * P], BF16, isOutput=False)
    y_d = nc.declare_dram_parameter("y", [T_SHARD, H], F32, isOutput=True)

    cc_ins = [nc.dram_tensor(f"cc_in{ch}", [TC, H], F32) for ch in range(NCH)]
    cc_outs = [nc.dram_tensor(f"cc_out{ch}", [TC // NCORES, H], F32)
               for ch in range(NCH)]

    with tile.TileContext(nc) as tc:
        with (
            tc.tile_pool(name="wres", bufs=1) as wres,
            tc.tile_pool(name="xtcp", bufs=3) as xtcp,
            tc.tile_pool(name="xtrp", bufs=2) as xtrp,
            tc.tile_pool(name="hgep", bufs=1) as hgep,
            tc.tile_pool(name="hswp", bufs=4) as hswp,
            tc.tile_pool(name="actp", bufs=2) as actp,
            tc.tile_pool(name="outp", bufs=2) as outp,
            tc.tile_pool(name="rtp", bufs=2) as rtp,
            tc.tile_pool(name="ps_tr", bufs=1, space="PSUM") as ps_tr,
            tc.tile_pool(name="ps_r", bufs=1, space="PSUM") as ps_r,
            tc.tile_pool(name="ps_g", bufs=2, space="PSUM") as ps_g,
            tc.tile_pool(name="ps_u", bufs=2, space="PSUM") as ps_u,
            tc.tile_pool(name="ps_d", bufs=1, space="PSUM") as ps_d,
        ):
            # ---------- constants / small weights ----------
            ident = wres.tile([P, P], F32, tag="ident")
            make_identity(nc, ident[:])

            # router weights first — the PE's first work is router(0)
            wr1_sb = wres.tile([P, NH * E], BF16, tag="wr1")
            nc.sync.dma_start(wr1_sb[:], wr1_d[:])
            wr2_sb = wres.tile([P, NH * E], BF16, tag="wr2")
            nc.sync.dma_start(wr2_sb[:], wr2_d[:])

            # per-chunk feature-major x + residual (bf16), contiguous DMAs.
            # Loaded as per-h-slice DMAs so consumers of early h slices can
            # start before the whole chunk has landed.
            def load_xt(ch):
                xtc = xtcp.tile([P, NH * TC], BF16, tag="xtc", name="xtc")
                xtr = xtrp.tile([P, NH * TC], BF16, tag="xtr", name="xtr")
                for h in range(NH):
                    s = slice(h * TC, (h + 1) * TC)
                    nc.sync.dma_start(xtc[:, s], xtb_d[ch][:, s])
                    nc.sync.dma_start(xtr[:, s], xtr_d[ch][:, s])
                return xtc, xtr

            x_chunks = {0: load_xt(0), 1: load_xt(1)}

            # selector constant for per-expert weight-row broadcast; lives at
            # partitions 64:68 to match hswe's routing-weight rows (matmul
            # operands must share a base partition)
            sel_bf = wres.tile([P, E_LOC * P], BF16, tag="sel")
            nc.sync.dma_start(sel_bf[I_SH:I_SH + E_LOC, :], sel_d[:])

            # biases
            br_sb = wres.tile([E, 1], F32, tag="br")
            nc.sync.dma_start(br_sb[:], br_d[:])
            bg_sb = wres.tile([P, E_LOC * NI], F32, tag="bg")
            nc.sync.dma_start(bg_sb[:], bg_d[:])
            bu_sb = wres.tile([P, E_LOC * NI], F32, tag="bu")
            nc.sync.dma_start(bu_sb[:], bu_d[:])
            bgs_sb = wres.tile([I_SH, 1], F32, tag="bgs")
            nc.sync.dma_start(bgs_sb[:], bgs_d[:])
            bus_sb = wres.tile([I_SH, 1], F32, tag="bus")
            nc.sync.dma_start(bus_sb[:], bus_d[:])

            # ---------- resident expert weights (direct bf16 loads) ----------
            wg_bf = {}
            wu_bf = {}
            wd_bf = {}
            for e in range(E_LOC):
                for name, dram, store in (("wg", wg_d, wg_bf), ("wu", wu_d, wu_bf)):
                    res = wres.tile([P, NH * I], BF16, tag=f"{name}{e}")
                    nc.scalar.dma_start(res[:], dram[e])
                    store[e] = res
                res = wres.tile([P, NI * H], BF16, tag=f"wd{e}")
                nc.scalar.dma_start(res[:], wd_d[e])
                wd_bf[e] = res
            wgu_sb = wres.tile([P, NH * 2 * I_SH], BF16, tag="wgu")
            nc.scalar.dma_start(wgu_sb[:], wgu_d[:])
            # stacked [bias5 rows 0:5 ; shared-down rows 5:69] moving operand
            wdsb5_sb = wres.tile([E_LOC + 1 + I_SH, H], BF16, tag="wdsb5")
            nc.scalar.dma_start(wdsb5_sb[:], wdsb5_d[:])

            def router(ch, xtc, xtr):
                """Returns hswe [69, TC]: rows 0:4 expert routing weights,
                row 4 ones, rows 5:69 shared-expert hge (written later by
                experts())."""
                t0 = ch * TC
                hswe = hswp.tile([I_SH + E_LOC + 1, TC], BF16, tag="hswe",
                                 name="hswe")
                # memset rows 64:69 at an aligned base; the routing-weight
                # copies below overwrite rows 64:68, leaving row 68 = ones
                nc.vector.memset(hswe[I_SH:I_SH + E_LOC + 1, :], 1.0)
                pr = ps_r.tile([E, TC], F32, tag="r", name="pr")
                for h in range(NH):
                    nc.tensor.matmul(pr[:], wr1_sb[:, h * E:(h + 1) * E],
                                     xtc[:, h * TC:(h + 1) * TC],
                                     start=(h == 0), stop=False)
                    nc.tensor.matmul(pr[:], wr1_sb[:, h * E:(h + 1) * E],
                                     xtr[:, h * TC:(h + 1) * TC],
                                     start=False, stop=False)
                    nc.tensor.matmul(pr[:], wr2_sb[:, h * E:(h + 1) * E],
                                     xtc[:, h * TC:(h + 1) * TC],
                                     start=False, stop=(h == NH - 1))
                logits_fm = rtp.tile([E, TC], F32, tag="logits_fm", bufs=1)
                nc.scalar.activation(logits_fm[:], pr[:], AF.Identity,
                                     bias=br_sb[:, 0:1])
                # transpose to token-major [128, 4, 32]
                logits_tm = rtp.tile([P, NJ, E], F32, tag="logits_tm")
                for j in range(NJ):
                    pt = ps_tr.tile([P, E], F32, tag="tr", name="ptl")
                    nc.tensor.transpose(pt[:], logits_fm[:, j * P:(j + 1) * P],
                                        ident[0:E, 0:E])
                    nc.vector.tensor_copy(logits_tm[:, j, :], pt[:])
                # top-8 threshold by iterative max extraction
                cur = rtp.tile([P, NJ, E], F32, tag="cur")
                nc.vector.tensor_copy(cur[:], logits_tm[:])
                mx = rtp.tile([P, NJ], F32, tag="mx")
                mask = rtp.tile([P, NJ, E], F32, tag="mask", bufs=1)
                for k in range(TOPK):
                    nc.vector.tensor_reduce(mx[:], cur[:], mybir.AxisListType.X,
                                            ALU.max)
                    if k < TOPK - 1:
                        mxb = mx[:].rearrange("p (f o) -> p f o", o=1).broadcast_to(
                            [P, NJ, E])
                        nc.vector.tensor_tensor(mask[:], cur[:], mxb, ALU.is_ge)
                        nc.vector.scalar_tensor_tensor(cur[:], mask[:], NEG, cur[:],
                                                       ALU.mult, ALU.add)
                # mask8 / normalized sigmoid weights
                aff = rtp.tile([P, NJ, E], F32, tag="aff")
                nc.scalar.activation(aff[:], logits_tm[:], AF.Sigmoid)
                thrb = mx[:].rearrange("p (f o) -> p f o", o=1).broadcast_to(
                    [P, NJ, E])
                nc.vector.tensor_tensor(mask[:], logits_tm[:], thrb, ALU.is_ge)
                nc.vector.tensor_tensor(aff[:], aff[:], mask[:], ALU.mult)
                den = rtp.tile([P, NJ], F32, tag="den")
                nc.vector.tensor_reduce(den[:], aff[:], mybir.AxisListType.X, ALU.add)
                rec = rtp.tile([P, NJ], F32, tag="rec")
                nc.vector.reciprocal(rec[:], den[:])
                recb = rec[:].rearrange("p (f o) -> p f o", o=1).broadcast_to(
                    [P, NJ, E])
                w_tm = rtp.tile([P, NJ, E], F32, tag="w_tm")
                nc.vector.tensor_tensor(w_tm[:], aff[:], recb, ALU.mult)
                # local expert weights, feature-major -> hswe rows 0..3 (bf16)
                for j in range(NJ):
                    pt = ps_tr.tile([E_LOC, P], F32, tag="tr", name="ptw")
                    nc.tensor.transpose(pt[:], w_tm[:, j, 0:E_LOC], ident[:])
                    nc.vector.tensor_copy(
                        hswe[I_SH:I_SH + E_LOC, j * P:(j + 1) * P], pt[:])
                return hswe

            def experts(ch, xtc, hswe):
                t0 = ch * TC
                # gate/up -> hge (bf16)
                hge = {}
                for e in range(E_LOC):
                    # broadcast token-weight row -> [128, TC] via selector matmul
                    pw = ps_r.tile([P, TC], F32, tag="r", name="pw")
                    nc.tensor.matmul(pw[:],
                                     sel_bf[I_SH:I_SH + E_LOC, e * P:(e + 1) * P],
                                     hswe[I_SH:I_SH + E_LOC, :],
                                     start=True, stop=True)
                    w_bc = actp.tile([P, TC], BF16, tag="w_bc", bufs=1)
                    nc.vector.tensor_copy(w_bc[:], pw[:])
                    for i in range(NI):
                        pg = ps_g.tile([P, TC], F32, tag="g")
                        pu = ps_u.tile([P, TC], F32, tag="u")
                        for h in range(NH):
                            nc.tensor.matmul(pg[:],
                                             wg_bf[e][:, h * I + i * P:h * I + (i + 1) * P],
                                             xtc[:, h * TC:(h + 1) * TC],
                                             start=(h == 0),
                                             stop=(h == NH - 1))
                        for h in range(NH):
                            nc.tensor.matmul(pu[:],
                                             wu_bf[e][:, h * I + i * P:h * I + (i + 1) * P],
                                             xtc[:, h * TC:(h + 1) * TC],
                                             start=(h == 0),
                                             stop=(h == NH - 1))
                        g_act = actp.tile([P, TC], F32, tag="g_act")
                        nc.scalar.activation(g_act[:], pg[:], AF.Silu,
                                             bias=bg_sb[:, e * NI + i:e * NI + i + 1])
                        u_w = actp.tile([P, TC], F32, tag="u_w")
                        nc.vector.scalar_tensor_tensor(
                            u_w[:], pu[:], bu_sb[:, e * NI + i:e * NI + i + 1],
                            w_bc[:], ALU.add, ALU.mult)
                        ht = hgep.tile([P, TC], BF16, tag=f"hge{e}_{i}", name="ht")
                        nc.vector.tensor_tensor(ht[:], g_act[:], u_w[:], ALU.mult)
                        hge[(e, i)] = ht

                # shared expert shard -> hge_s (bf16, 64 partitions).
                # gate and up are packed into one [H, 128] stationary block
                # (rows 0:64 gate, 64:128 up) so the PE array runs full-width.
                psgu = ps_g.tile([2 * I_SH, TC], F32, tag="g", name="psgu")
                for h in range(NH):
                    nc.tensor.matmul(
                        psgu[:], wgu_sb[:, h * 2 * I_SH:(h + 1) * 2 * I_SH],
                        xtc[:, h * TC:(h + 1) * TC],
                        start=(h == 0), stop=(h == NH - 1))
                gs = actp.tile([I_SH, TC], F32, tag="gs", bufs=1)
                nc.scalar.activation(gs[:], psgu[0:I_SH, :], AF.Silu,
                                     bias=bgs_sb[:, 0:1])
                nc.vector.scalar_tensor_tensor(
                    hswe[0:I_SH, :],
                    psgu[I_SH:2 * I_SH, :], bus_sb[:, 0:1], gs[:],
                    ALU.add, ALU.mult)

                # down projection, token-major output
                for j in range(NJ):
                    ts = t0 + j * P
                    out_sb = outp.tile([P, H], F32, tag="out")
                    for half in range(2):
                        hs0 = half * (H // 2)
                        pd = ps_d.tile([P, H // 2], F32, tag=f"d{half}",
                                       name=f"pd{half}")
                        m = 0
                        for e in range(E_LOC):
                            for i in range(NI):
                                nc.tensor.matmul(
                                    pd[:],
                                    hge[(e, i)][:, j * P:(j + 1) * P],
                                    wd_bf[e][:, i * H + hs0:i * H + hs0 + H // 2],
                                    start=(m == 0), stop=False)
                                m += 1
                        nc.tensor.matmul(pd[:],
                                         hswe[:, j * P:(j + 1) * P],
                                         wdsb5_sb[:, hs0:hs0 + H // 2],
                                         start=False, stop=True)
                        nc.vector.tensor_copy(out_sb[:, hs0:hs0 + H // 2], pd[:])
                    ch_i, off = divmod(ts, TC)
                    nc.gpsimd.dma_start(cc_ins[ch_i][off:off + P, :], out_sb[:])

            # ---------- main loop ----------
            def reduce_chunk(ch):
                nc.gpsimd.collective_compute(
                    "ReduceScatter",
                    ALU.add,
                    ins=[cc_ins[ch][:]],
                    outs=[cc_outs[ch][:]],
                    replica_groups=[list(range(NCORES))],
                )
                rows = TC // NCORES
                nc.scalar.dma_start(y_d[ch * rows:(ch + 1) * rows, :],
                                    cc_outs[ch][:])

            # routers 0-3 run up front: ~24 us of PE work that overlaps the
            # expert-weight DMA stream, instead of the PE stalling on wg0/wu0
            hswes = {0: router(0, *x_chunks[0]), 1: router(1, *x_chunks[1])}
            for ch in range(NCH):
                if ch + 2 < NCH:
                    x_chunks[ch + 2] = load_xt(ch + 2)
                xtc, _ = x_chunks.pop(ch)
                experts(ch, xtc, hswes.pop(ch))
                if ch + 2 < NCH:
                    hswes[ch + 2] = router(ch + 2, *x_chunks[ch + 2])
                reduce_chunk(ch)

    nc.finalize()
    return nc


def _pm(a, p=P):
    """[..., n*p, cols] -> partition-major [..., p, n*cols] with contiguous
    per-partition rows."""
    *lead, rows, cols = a.shape
    n = rows // p
    return np.ascontiguousarray(
        a.reshape(*lead, n, p, cols).swapaxes(-3, -2).reshape(*lead, p, n * cols))


def prep_inputs(inputs):
    """Split/replicate full inputs into 8 per-core input maps (layout + dtype
    prep only — bf16 casts, transposes, partition-major relayouts)."""
    import ml_dtypes

    bf16 = ml_dtypes.bfloat16
    hs = np.ascontiguousarray(np.asarray(inputs["hidden_states"], dtype=np.float32))
    x = hs.reshape(T, H)
    x_bf = x.astype(bf16)
    x_r = (x - x_bf.astype(np.float32)).astype(bf16)

    def xlayout(xt):
        # [H, T] -> [NCH, P, NH*TC]
        return np.ascontiguousarray(
            xt.reshape(NH, P, NCH, TC).transpose(2, 1, 0, 3).reshape(NCH, P, NH * TC))

    xtb = xlayout(np.ascontiguousarray(x_bf.T))
    xtr = xlayout(np.ascontiguousarray(x_r.T))
    Wr = np.asarray(inputs["Wr"], np.float32)
    br = np.asarray(inputs["br"], np.float32)
    Wg = np.asarray(inputs["Wg"], np.float32)
    bg = np.asarray(inputs["bg"], np.float32)
    Wu = np.asarray(inputs["Wu"], np.float32)
    bu = np.asarray(inputs["bu"], np.float32)
    Wd = np.asarray(inputs["Wd"], np.float32)
    bd = np.asarray(inputs["bd"], np.float32)
    Wg_s = np.asarray(inputs["Wg_s"], np.float32)
    bg_s = np.asarray(inputs["bg_s"], np.float32)
    Wu_s = np.asarray(inputs["Wu_s"], np.float32)
    bu_s = np.asarray(inputs["bu_s"], np.float32)
    Wd_s = np.asarray(inputs["Wd_s"], np.float32)
    bd_s = np.asarray(inputs["bd_s"], np.float32)

    sel = np.kron(np.eye(E_LOC, dtype=np.float32),
                  np.ones((1, P), dtype=np.float32)).astype(bf16)

    in_maps = []
    for c in range(NCORES):
        loc = list(range(c * E_LOC, (c + 1) * E_LOC))
        rest = [e for e in range(E) if e not in loc]
        perm = loc + rest
        sh = slice(c * I_SH, (c + 1) * I_SH)
        wr_f = np.ascontiguousarray(Wr[:, perm])
        wr1 = wr_f.astype(bf16)
        wr2 = (wr_f - wr1.astype(np.float32)).astype(bf16)
        wdsb5 = np.concatenate(
            [Wd_s[sh, :], bd[loc],
             (bd_s if c == 0 else np.zeros_like(bd_s))[None, :]],
            axis=0).astype(bf16)
        in_maps.append({
            "xtb": xtb,
            "xtr": xtr,
            "wr1": _pm(wr1),
            "wr2": _pm(wr2),
            "br": np.ascontiguousarray(br[perm])[:, None],
            "wg": _pm(Wg[loc].astype(bf16)),
            "wu": _pm(Wu[loc].astype(bf16)),
            "wd": _pm(Wd[loc].astype(bf16)),
            "bg": np.ascontiguousarray(
                bg[loc].reshape(E_LOC, NI, P).transpose(2, 0, 1).reshape(P, E_LOC * NI)),
            "bu": np.ascontiguousarray(
                bu[loc].reshape(E_LOC, NI, P).transpose(2, 0, 1).reshape(P, E_LOC * NI)),

            "wgu": _pm(np.concatenate([Wg_s[:, sh], Wu_s[:, sh]],
                                       axis=1).astype(bf16)),
            "wdsb5": np.ascontiguousarray(wdsb5),
            "bgs": np.ascontiguousarray(bg_s[sh])[:, None],
            "bus": np.ascontiguousarray(bu_s[sh])[:, None],
            "sel": sel,
        })
    return in_maps


def assemble_output(results):
    """Reassemble [T, H]: 8 per-chunk ReduceScatters give core c the rows
    [ch*512 + c*64 : ch*512 + (c+1)*64] in y rows [ch*64:(ch+1)*64]."""
    ys = np.stack([results[c]["y"] for c in range(NCORES)])      # [8, 512, H]
    rows = TC // NCORES                                           # 64
    return np.ascontiguousarray(
        ys.reshape(NCORES, NCH, rows, H).transpose(1, 0, 2, 3).reshape(T, H))


_CACHE = {}


def get_runner():
    """Build + jit once; returns run(in_maps) -> list of per-core output dicts."""
    if "run" in _CACHE:
        return _CACHE["run"]
    import jax
    from jax.sharding import Mesh, PartitionSpec
    from jax.experimental.shard_map import shard_map
    from concourse import bass2jax

    nc = build_nc()
    bass2jax.install_neuronx_cc_hook()

    in_names = []
    out_names = []
    out_avals = []
    partition_name = nc.partition_id_tensor.name if nc.partition_id_tensor else None
    for alloc in nc.m.functions[0].allocations:
        if not isinstance(alloc, mybir.MemoryLocationSet):
            continue
        name = alloc.memorylocations[0].name
        if alloc.kind == "ExternalInput":
            if name != partition_name:
                in_names.append(name)
        elif alloc.kind == "ExternalOutput":
            out_names.append(name)
            out_avals.append(
                jax.core.ShapedArray(tuple(alloc.tensor_shape),
                                     mybir.dt.np(alloc.dtype)))
    n_params = len(in_names)
    n_outs = len(out_names)
    all_names = in_names + out_names + ([partition_name] if partition_name else [])
    donate = tuple(range(n_params, n_params + n_outs))

    def _body(*args):
        operands = list(args)
        if partition_name is not None:
            operands.append(bass2jax.partition_id_tensor())
        return tuple(bass2jax._bass_exec_p.bind(
            *operands,
            out_avals=tuple(out_avals),
            in_names=tuple(all_names),
            out_names=tuple(out_names),
            lowering_input_output_aliases=(),
            sim_require_finite=True,
            sim_require_nnan=True,
            nc=nc,
        ))

    devices = jax.devices()[:NCORES]
    mesh = Mesh(np.asarray(devices), ("core",))
    in_specs = (PartitionSpec("core"),) * (n_params + n_outs)
    out_specs = (PartitionSpec("core"),) * n_outs
    sharded = jax.jit(
        shard_map(_body, mesh=mesh, in_specs=in_specs, out_specs=out_specs,
                  check_rep=False),
        donate_argnums=donate, keep_unused=True)

    def run(in_maps, dev_inputs=None):
        if dev_inputs is None:
            dev_inputs = [
                np.concatenate([np.asarray(in_maps[c][n]) for c in range(NCORES)],
                               axis=0)
                for n in in_names
            ]
        zeros = [np.zeros((NCORES * a.shape[0], *a.shape[1:]), a.dtype)
                 for a in out_avals]
        outs = sharded(*dev_inputs, *zeros)
        return [
            {name: np.asarray(outs[i]).reshape(NCORES, *out_avals[i].shape)[c]
             for i, name in enumerate(out_names)}
            for c in range(NCORES)
        ]

    _CACHE["run"] = run
    _CACHE["meta"] = (in_names, out_names, out_avals, sharded, mesh)
    return run


def kernel(**inputs) -> np.ndarray:
    run = get_runner()
    in_maps = prep_inputs(inputs)
    results = run(in_maps)
    return assemble_output(results).reshape(B, S, H).astype(np.float32)


# revision 23
# speedup vs baseline: 1.0935x; 1.0461x over previous
"""DeepSeekV3-style MoE layer (1 MoE block) on 8 Trainium2 NeuronCores.

Sharding: expert-parallel. Each core owns 4 of the 32 routed experts and a
64-wide shard of the shared expert's intermediate dim. The router is
replicated (router weight columns are permuted per-core so the local experts
always sit in columns 0..3 — top-k and sigmoid are permutation invariant).
Partial outputs are combined with one on-device ReduceScatter; the host
reassembles the 8 output shards.

Host-side prep (layout/dtype only, in prep_inputs): x is cast to bf16 plus a
bf16 residual (router accuracy), transposed to feature-major and laid out
partition-major per chunk so every device DMA is contiguous 8 KiB rows; all
weights are pre-cast to bf16 (router as a split-bf16 pair w1+w2 ~= Wr fp32)
and laid out partition-major. The device kernel is pure matmul + top-k +
elementwise — no on-device transposes, casts, or strided DMA.

Per-core device pipeline (feature-major activations):
  - router runs in split-bf16 with ~fp32 accuracy:
    logits = w1.x1 + w1.x2 + w2.x1 accumulated in one fp32 PSUM group
  - top-8 selection on logits via iterative max extraction (sigmoid is
    monotonic so logit order == affinity order), normalized sigmoid weights
  - bf16 gate/up matmuls -> silu(g+bg) * (u+bu) * token_weight -> bf16 hge
  - down-projection with hge as the stationary operand so the PSUM output is
    token-major [128 tokens x H], accumulating all 4 experts + shared expert
    + bias trick ([w_e rows; ones] @ [bd_e rows; bd_shared]) in one group.
"""

import sys

sys.path.insert(0, "/opt/trn_rl_repo")

import numpy as np

import concourse.bacc as bacc
import concourse.bass as bass
import concourse.mybir as mybir
import concourse.tile as tile
from concourse.masks import make_identity

F32 = mybir.dt.float32
BF16 = mybir.dt.bfloat16
AF = mybir.ActivationFunctionType
ALU = mybir.AluOpType

H, I, E, TOPK = 1024, 512, 32, 8
B, S = 4, 1024
T = B * S
NCORES = 8
E_LOC = E // NCORES          # 4 routed experts per core
I_SH = I // NCORES           # 64-wide shared-expert shard per core
P = 128
TC = 512                     # token chunk
NCH = T // TC                # 8 chunks
NH = H // P                  # 8 hidden k-tiles
NI = I // P                  # 4 intermediate tiles
NJ = TC // P                 # 4 token tiles per chunk
T_SHARD = T // NCORES        # 512 rows per core after ReduceScatter
NEG = -1.0e30


def build_nc():
    nc = bacc.Bacc(None, target_bir_lowering=False, num_devices=NCORES)

    # all DRAM layouts are partition-major with contiguous per-partition rows
    xtb_d = nc.declare_dram_parameter("xtb", [NCH, P, NH * TC], BF16, isOutput=False)
    xtr_d = nc.declare_dram_parameter("xtr", [NCH, P, NH * TC], BF16, isOutput=False)
    wr1_d = nc.declare_dram_parameter("wr1", [P, NH * E], BF16, isOutput=False)
    wr2_d = nc.declare_dram_parameter("wr2", [P, NH * E], BF16, isOutput=False)
    br_d = nc.declare_dram_parameter("br", [E, 1], F32, isOutput=False)
    wg_d = nc.declare_dram_parameter("wg", [E_LOC, P, NH * I], BF16, isOutput=False)
    wu_d = nc.declare_dram_parameter("wu", [E_LOC, P, NH * I], BF16, isOutput=False)
    wd_d = nc.declare_dram_parameter("wd", [E_LOC, P, NI * H], BF16, isOutput=False)
    bg_d = nc.declare_dram_parameter("bg", [P, E_LOC * NI], F32, isOutput=False)
    bu_d = nc.declare_dram_parameter("bu", [P, E_LOC * NI], F32, isOutput=False)

    wgu_d = nc.declare_dram_parameter("wgu", [P, NH * 2 * I_SH], BF16, isOutput=False)
    wdsb5_d = nc.declare_dram_parameter("wdsb5", [E_LOC + 1 + I_SH, H], BF16,
                                        isOutput=False)
    bgs_d = nc.declare_dram_parameter("bgs", [I_SH, 1], F32, isOutput=False)
    bus_d = nc.declare_dram_parameter("bus", [I_SH, 1], F32, isOutput=False)
    sel_d = nc.declare_dram_parameter("sel", [E_LOC, E_LOC * P], BF16, isOutput=False)
    y_d = nc.declare_dram_parameter("y", [T_SHARD, H], F32, isOutput=True)

    cc_ins = [nc.dram_tensor(f"cc_in{ch}", [TC, H], F32) for ch in range(NCH)]
    cc_outs = [nc.dram_tensor(f"cc_out{ch}", [TC // NCORES, H], F32)
               for ch in range(NCH)]

    with tile.TileContext(nc) as tc:
        with (
            tc.tile_pool(name="wres", bufs=1) as wres,
            tc.tile_pool(name="xtcp", bufs=3) as xtcp,
            tc.tile_pool(name="xtrp", bufs=2) as xtrp,
            tc.tile_pool(name="hgep", bufs=1) as hgep,
            tc.tile_pool(name="hswp", bufs=4) as hswp,
            tc.tile_pool(name="actp", bufs=2) as actp,
            tc.tile_pool(name="outp", bufs=2) as outp,
            tc.tile_pool(name="rtp", bufs=2) as rtp,
            tc.tile_pool(name="ps_tr", bufs=1, space="PSUM") as ps_tr,
            tc.tile_pool(name="ps_r", bufs=1, space="PSUM") as ps_r,
            tc.tile_pool(name="ps_g", bufs=2, space="PSUM") as ps_g,
            tc.tile_pool(name="ps_u", bufs=2, space="PSUM") as ps_u,
            tc.tile_pool(name="ps_d", bufs=1, space="PSUM") as ps_d,
        ):
            # ---------- constants / small weights ----------
            ident = wres.tile([P, P], F32, tag="ident")
            make_identity(nc, ident[:])

            # router weights first — the PE's first work is router(0)
            wr1_sb = wres.tile([P, NH * E], BF16, tag="wr1")
            nc.sync.dma_start(wr1_sb[:], wr1_d[:])
            wr2_sb = wres.tile([P, NH * E], BF16, tag="wr2")
            nc.sync.dma_start(wr2_sb[:], wr2_d[:])

            # per-chunk feature-major x + residual (bf16), contiguous DMAs.
            # Loaded as per-h-slice DMAs so consumers of early h slices can
            # start before the whole chunk has landed.
            def load_xt(ch):
                xtc = xtcp.tile([P, NH * TC], BF16, tag="xtc", name="xtc")
                xtr = xtrp.tile([P, NH * TC], BF16, tag="xtr", name="xtr")
                for h in range(NH):
                    s = slice(h * TC, (h + 1) * TC)
                    nc.sync.dma_start(xtc[:, s], xtb_d[ch][:, s])
                    nc.sync.dma_start(xtr[:, s], xtr_d[ch][:, s])
                return xtc, xtr

            x_chunks = {0: load_xt(0), 1: load_xt(1)}

            # selector constant for per-expert weight-row broadcast; lives at
            # partitions 64:68 to match hswe's routing-weight rows (matmul
            # operands must share a base partition)
            sel_bf = wres.tile([P, E_LOC * P], BF16, tag="sel")
            nc.sync.dma_start(sel_bf[I_SH:I_SH + E_LOC, :], sel_d[:])

            # biases
            br_sb = wres.tile([E, 1], F32, tag="br")
            nc.sync.dma_start(br_sb[:], br_d[:])
            bg_sb = wres.tile([P, E_LOC * NI], F32, tag="bg")
            nc.sync.dma_start(bg_sb[:], bg_d[:])
            bu_sb = wres.tile([P, E_LOC * NI], F32, tag="bu")
            nc.sync.dma_start(bu_sb[:], bu_d[:])
            bgs_sb = wres.tile([I_SH, 1], F32, tag="bgs")
            nc.sync.dma_start(bgs_sb[:], bgs_d[:])
            bus_sb = wres.tile([I_SH, 1], F32, tag="bus")
            nc.sync.dma_start(bus_sb[:], bus_d[:])

            # ---------- resident expert weights (direct bf16 loads) ----------
            wg_bf = {}
            wu_bf = {}
            wd_bf = {}
            # gate/up weights first (gate the first experts() chunks); the
            # down weights aren't needed until the first down-projection
            for e in range(E_LOC):
                for name, dram, store in (("wg", wg_d, wg_bf), ("wu", wu_d, wu_bf)):
                    res = wres.tile([P, NH * I], BF16, tag=f"{name}{e}")
                    nc.scalar.dma_start(res[:], dram[e])
                    store[e] = res
            for e in range(E_LOC):
                res = wres.tile([P, NI * H], BF16, tag=f"wd{e}")
                nc.scalar.dma_start(res[:], wd_d[e])
                wd_bf[e] = res
            wgu_sb = wres.tile([P, NH * 2 * I_SH], BF16, tag="wgu")
            nc.scalar.dma_start(wgu_sb[:], wgu_d[:])
            # stacked [bias5 rows 0:5 ; shared-down rows 5:69] moving operand
            wdsb5_sb = wres.tile([E_LOC + 1 + I_SH, H], BF16, tag="wdsb5")
            nc.scalar.dma_start(wdsb5_sb[:], wdsb5_d[:])

            def router(ch, xtc, xtr):
                """Returns hswe [69, TC]: rows 0:4 expert routing weights,
                row 4 ones, rows 5:69 shared-expert hge (written later by
                experts())."""
                t0 = ch * TC
                hswe = hswp.tile([I_SH + E_LOC + 1, TC], BF16, tag="hswe",
                                 name="hswe")
                # memset rows 64:69 at an aligned base; the routing-weight
                # copies below overwrite rows 64:68, leaving row 68 = ones
                nc.vector.memset(hswe[I_SH:I_SH + E_LOC + 1, :], 1.0)
                pr = ps_r.tile([E, TC], F32, tag="r", name="pr")
                for h in range(NH):
                    nc.tensor.matmul(pr[:], wr1_sb[:, h * E:(h + 1) * E],
                                     xtc[:, h * TC:(h + 1) * TC],
                                     start=(h == 0), stop=False)
                    nc.tensor.matmul(pr[:], wr1_sb[:, h * E:(h + 1) * E],
                                     xtr[:, h * TC:(h + 1) * TC],
                                     start=False, stop=False)
                    nc.tensor.matmul(pr[:], wr2_sb[:, h * E:(h + 1) * E],
                                     xtc[:, h * TC:(h + 1) * TC],
                                     start=False, stop=(h == NH - 1))
                logits_fm = rtp.tile([E, TC], F32, tag="logits_fm", bufs=1)
                nc.scalar.activation(logits_fm[:], pr[:], AF.Identity,
                                     bias=br_sb[:, 0:1])
                # transpose to token-major [128, 4, 32]
                logits_tm = rtp.tile([P, NJ, E], F32, tag="logits_tm")
                for j in range(NJ):
                    pt = ps_tr.tile([P, E], F32, tag="tr", name="ptl")
                    nc.tensor.transpose(pt[:], logits_fm[:, j * P:(j + 1) * P],
                                        ident[0:E, 0:E])
                    nc.vector.tensor_copy(logits_tm[:, j, :], pt[:])
                # top-8 threshold by iterative max extraction
                cur = rtp.tile([P, NJ, E], F32, tag="cur")
                nc.vector.tensor_copy(cur[:], logits_tm[:])
                mx = rtp.tile([P, NJ], F32, tag="mx")
                mask = rtp.tile([P, NJ, E], F32, tag="mask", bufs=1)
                for k in range(TOPK):
                    nc.vector.tensor_reduce(mx[:], cur[:], mybir.AxisListType.X,
                                            ALU.max)
                    if k < TOPK - 1:
                        mxb = mx[:].rearrange("p (f o) -> p f o", o=1).broadcast_to(
                            [P, NJ, E])
                        nc.vector.tensor_tensor(mask[:], cur[:], mxb, ALU.is_ge)
                        nc.vector.scalar_tensor_tensor(cur[:], mask[:], NEG, cur[:],
                                                       ALU.mult, ALU.add)
                # mask8 / normalized sigmoid weights
                aff = rtp.tile([P, NJ, E], F32, tag="aff")
                nc.scalar.activation(aff[:], logits_tm[:], AF.Sigmoid)
                thrb = mx[:].rearrange("p (f o) -> p f o", o=1).broadcast_to(
                    [P, NJ, E])
                nc.vector.tensor_tensor(mask[:], logits_tm[:], thrb, ALU.is_ge)
                nc.vector.tensor_tensor(aff[:], aff[:], mask[:], ALU.mult)
                den = rtp.tile([P, NJ], F32, tag="den")
                nc.vector.tensor_reduce(den[:], aff[:], mybir.AxisListType.X, ALU.add)
                rec = rtp.tile([P, NJ], F32, tag="rec")
                nc.vector.reciprocal(rec[:], den[:])
                recb = rec[:].rearrange("p (f o) -> p f o", o=1).broadcast_to(
                    [P, NJ, E])
                w_tm = rtp.tile([P, NJ, E], F32, tag="w_tm")
                nc.vector.tensor_tensor(w_tm[:], aff[:], recb, ALU.mult)
                # local expert weights, feature-major -> hswe rows 0..3 (bf16)
                for j in range(NJ):
                    pt = ps_tr.tile([E_LOC, P], F32, tag="tr", name="ptw")
                    nc.tensor.transpose(pt[:], w_tm[:, j, 0:E_LOC], ident[:])
                    nc.vector.tensor_copy(
                        hswe[I_SH:I_SH + E_LOC, j * P:(j + 1) * P], pt[:])
                return hswe

            def experts(ch, xtc, hswe):
                t0 = ch * TC
                # gate/up -> hge (bf16)
                hge = {}
                for e in range(E_LOC):
                    # broadcast token-weight row -> [128, TC] via selector matmul
                    pw = ps_r.tile([P, TC], F32, tag="r", name="pw")
                    nc.tensor.matmul(pw[:],
                                     sel_bf[I_SH:I_SH + E_LOC, e * P:(e + 1) * P],
                                     hswe[I_SH:I_SH + E_LOC, :],
                                     start=True, stop=True)
                    w_bc = actp.tile([P, TC], BF16, tag="w_bc", bufs=1)
                    nc.vector.tensor_copy(w_bc[:], pw[:])
                    for i in range(NI):
                        pg = ps_g.tile([P, TC], F32, tag="g")
                        pu = ps_u.tile([P, TC], F32, tag="u")
                        for h in range(NH):
                            nc.tensor.matmul(pg[:],
                                             wg_bf[e][:, h * I + i * P:h * I + (i + 1) * P],
                                             xtc[:, h * TC:(h + 1) * TC],
                                             start=(h == 0),
                                             stop=(h == NH - 1))
                        for h in range(NH):
                            nc.tensor.matmul(pu[:],
                                             wu_bf[e][:, h * I + i * P:h * I + (i + 1) * P],
                                             xtc[:, h * TC:(h + 1) * TC],
                                             start=(h == 0),
                                             stop=(h == NH - 1))
                        g_act = actp.tile([P, TC], F32, tag="g_act")
                        nc.scalar.activation(g_act[:], pg[:], AF.Silu,
                                             bias=bg_sb[:, e * NI + i:e * NI + i + 1])
                        u_w = actp.tile([P, TC], F32, tag="u_w")
                        nc.vector.scalar_tensor_tensor(
                            u_w[:], pu[:], bu_sb[:, e * NI + i:e * NI + i + 1],
                            w_bc[:], ALU.add, ALU.mult)
                        ht = hgep.tile([P, TC], BF16, tag=f"hge{e}_{i}", name="ht")
                        nc.vector.tensor_tensor(ht[:], g_act[:], u_w[:], ALU.mult)
                        hge[(e, i)] = ht

                # shared expert shard -> hge_s (bf16, 64 partitions).
                # gate and up are packed into one [H, 128] stationary block
                # (rows 0:64 gate, 64:128 up) so the PE array runs full-width.
                psgu = ps_g.tile([2 * I_SH, TC], F32, tag="g", name="psgu")
                for h in range(NH):
                    nc.tensor.matmul(
                        psgu[:], wgu_sb[:, h * 2 * I_SH:(h + 1) * 2 * I_SH],
                        xtc[:, h * TC:(h + 1) * TC],
                        start=(h == 0), stop=(h == NH - 1))
                gs = actp.tile([I_SH, TC], F32, tag="gs", bufs=1)
                nc.scalar.activation(gs[:], psgu[0:I_SH, :], AF.Silu,
                                     bias=bgs_sb[:, 0:1])
                nc.vector.scalar_tensor_tensor(
                    hswe[0:I_SH, :],
                    psgu[I_SH:2 * I_SH, :], bus_sb[:, 0:1], gs[:],
                    ALU.add, ALU.mult)

                # down projection, token-major output
                for j in range(NJ):
                    ts = t0 + j * P
                    out_sb = outp.tile([P, H], F32, tag="out")
                    for half in range(2):
                        hs0 = half * (H // 2)
                        pd = ps_d.tile([P, H // 2], F32, tag=f"d{half}",
                                       name=f"pd{half}")
                        m = 0
                        for e in range(E_LOC):
                            for i in range(NI):
                                nc.tensor.matmul(
                                    pd[:],
                                    hge[(e, i)][:, j * P:(j + 1) * P],
                                    wd_bf[e][:, i * H + hs0:i * H + hs0 + H // 2],
                                    start=(m == 0), stop=False)
                                m += 1
                        nc.tensor.matmul(pd[:],
                                         hswe[:, j * P:(j + 1) * P],
                                         wdsb5_sb[:, hs0:hs0 + H // 2],
                                         start=False, stop=True)
                        nc.vector.tensor_copy(out_sb[:, hs0:hs0 + H // 2], pd[:])
                    ch_i, off = divmod(ts, TC)
                    nc.gpsimd.dma_start(cc_ins[ch_i][off:off + P, :], out_sb[:])

            # ---------- main loop ----------
            def reduce_chunk(ch):
                nc.gpsimd.collective_compute(
                    "ReduceScatter",
                    ALU.add,
                    ins=[cc_ins[ch][:]],
                    outs=[cc_outs[ch][:]],
                    replica_groups=[list(range(NCORES))],
                )
                rows = TC // NCORES
                nc.scalar.dma_start(y_d[ch * rows:(ch + 1) * rows, :],
                                    cc_outs[ch][:])

            # routers 0-3 run up front: ~24 us of PE work that overlaps the
            # expert-weight DMA stream, instead of the PE stalling on wg0/wu0
            hswes = {0: router(0, *x_chunks[0]), 1: router(1, *x_chunks[1])}
            for ch in range(NCH):
                if ch + 2 < NCH:
                    x_chunks[ch + 2] = load_xt(ch + 2)
                xtc, _ = x_chunks.pop(ch)
                experts(ch, xtc, hswes.pop(ch))
                if ch + 2 < NCH:
                    hswes[ch + 2] = router(ch + 2, *x_chunks[ch + 2])
                reduce_chunk(ch)

    nc.finalize()
    return nc


def _pm(a, p=P):
    """[..., n*p, cols] -> partition-major [..., p, n*cols] with contiguous
    per-partition rows."""
    *lead, rows, cols = a.shape
    n = rows // p
    return np.ascontiguousarray(
        a.reshape(*lead, n, p, cols).swapaxes(-3, -2).reshape(*lead, p, n * cols))


def prep_inputs(inputs):
    """Split/replicate full inputs into 8 per-core input maps (layout + dtype
    prep only — bf16 casts, transposes, partition-major relayouts)."""
    import ml_dtypes

    bf16 = ml_dtypes.bfloat16
    hs = np.ascontiguousarray(np.asarray(inputs["hidden_states"], dtype=np.float32))
    x = hs.reshape(T, H)
    x_bf = x.astype(bf16)
    x_r = (x - x_bf.astype(np.float32)).astype(bf16)

    def xlayout(xt):
        # [H, T] -> [NCH, P, NH*TC]
        return np.ascontiguousarray(
            xt.reshape(NH, P, NCH, TC).transpose(2, 1, 0, 3).reshape(NCH, P, NH * TC))

    xtb = xlayout(np.ascontiguousarray(x_bf.T))
    xtr = xlayout(np.ascontiguousarray(x_r.T))
    Wr = np.asarray(inputs["Wr"], np.float32)
    br = np.asarray(inputs["br"], np.float32)
    Wg = np.asarray(inputs["Wg"], np.float32)
    bg = np.asarray(inputs["bg"], np.float32)
    Wu = np.asarray(inputs["Wu"], np.float32)
    bu = np.asarray(inputs["bu"], np.float32)
    Wd = np.asarray(inputs["Wd"], np.float32)
    bd = np.asarray(inputs["bd"], np.float32)
    Wg_s = np.asarray(inputs["Wg_s"], np.float32)
    bg_s = np.asarray(inputs["bg_s"], np.float32)
    Wu_s = np.asarray(inputs["Wu_s"], np.float32)
    bu_s = np.asarray(inputs["bu_s"], np.float32)
    Wd_s = np.asarray(inputs["Wd_s"], np.float32)
    bd_s = np.asarray(inputs["bd_s"], np.float32)

    sel = np.kron(np.eye(E_LOC, dtype=np.float32),
                  np.ones((1, P), dtype=np.float32)).astype(bf16)

    in_maps = []
    for c in range(NCORES):
        loc = list(range(c * E_LOC, (c + 1) * E_LOC))
        rest = [e for e in range(E) if e not in loc]
        perm = loc + rest
        sh = slice(c * I_SH, (c + 1) * I_SH)
        wr_f = np.ascontiguousarray(Wr[:, perm])
        wr1 = wr_f.astype(bf16)
        wr2 = (wr_f - wr1.astype(np.float32)).astype(bf16)
        wdsb5 = np.concatenate(
            [Wd_s[sh, :], bd[loc],
             (bd_s if c == 0 else np.zeros_like(bd_s))[None, :]],
            axis=0).astype(bf16)
        in_maps.append({
            "xtb": xtb,
            "xtr": xtr,
            "wr1": _pm(wr1),
            "wr2": _pm(wr2),
            "br": np.ascontiguousarray(br[perm])[:, None],
            "wg": _pm(Wg[loc].astype(bf16)),
            "wu": _pm(Wu[loc].astype(bf16)),
            "wd": _pm(Wd[loc].astype(bf16)),
            "bg": np.ascontiguousarray(
                bg[loc].reshape(E_LOC, NI, P).transpose(2, 0, 1).reshape(P, E_LOC * NI)),
            "bu": np.ascontiguousarray(
                bu[loc].reshape(E_LOC, NI, P).transpose(2, 0, 1).reshape(P, E_LOC * NI)),

            "wgu": _pm(np.concatenate([Wg_s[:, sh], Wu_s[:, sh]],
                                       axis=1).astype(bf16)),
            "wdsb5": np.ascontiguousarray(wdsb5),
            "bgs": np.ascontiguousarray(bg_s[sh])[:, None],
            "bus": np.ascontiguousarray(bu_s[sh])[:, None],
            "sel": sel,
        })
    return in_maps


def assemble_output(results):
    """Reassemble [T, H]: 8 per-chunk ReduceScatters give core c the rows
    [ch*512 + c*64 : ch*512 + (c+1)*64] in y rows [ch*64:(ch+1)*64]."""
    ys = np.stack([results[c]["y"] for c in range(NCORES)])      # [8, 512, H]
    rows = TC // NCORES                                           # 64
    return np.ascontiguousarray(
        ys.reshape(NCORES, NCH, rows, H).transpose(1, 0, 2, 3).reshape(T, H))


_CACHE = {}


def get_runner():
    """Build + jit once; returns run(in_maps) -> list of per-core output dicts."""
    if "run" in _CACHE:
        return _CACHE["run"]
    import jax
    from jax.sharding import Mesh, PartitionSpec
    from jax.experimental.shard_map import shard_map
    from concourse import bass2jax

    nc = build_nc()
    bass2jax.install_neuronx_cc_hook()

    in_names = []
    out_names = []
    out_avals = []
    partition_name = nc.partition_id_tensor.name if nc.partition_id_tensor else None
    for alloc in nc.m.functions[0].allocations:
        if not isinstance(alloc, mybir.MemoryLocationSet):
            continue
        name = alloc.memorylocations[0].name
        if alloc.kind == "ExternalInput":
            if name != partition_name:
                in_names.append(name)
        elif alloc.kind == "ExternalOutput":
            out_names.append(name)
            out_avals.append(
                jax.core.ShapedArray(tuple(alloc.tensor_shape),
                                     mybir.dt.np(alloc.dtype)))
    n_params = len(in_names)
    n_outs = len(out_names)
    all_names = in_names + out_names + ([partition_name] if partition_name else [])
    donate = tuple(range(n_params, n_params + n_outs))

    def _body(*args):
        operands = list(args)
        if partition_name is not None:
            operands.append(bass2jax.partition_id_tensor())
        return tuple(bass2jax._bass_exec_p.bind(
            *operands,
            out_avals=tuple(out_avals),
            in_names=tuple(all_names),
            out_names=tuple(out_names),
            lowering_input_output_aliases=(),
            sim_require_finite=True,
            sim_require_nnan=True,
            nc=nc,
        ))

    devices = jax.devices()[:NCORES]
    mesh = Mesh(np.asarray(devices), ("core",))
    in_specs = (PartitionSpec("core"),) * (n_params + n_outs)
    out_specs = (PartitionSpec("core"),) * n_outs
    sharded = jax.jit(
        shard_map(_body, mesh=mesh, in_specs=in_specs, out_specs=out_specs,
                  check_rep=False),
        donate_argnums=donate, keep_unused=True)

    def run(in_maps, dev_inputs=None):
        if dev_inputs is None:
            dev_inputs = [
                np.concatenate([np.asarray(in_maps[c][n]) for c in range(NCORES)],
                               axis=0)
                for n in in_names
            ]
        zeros = [np.zeros((NCORES * a.shape[0], *a.shape[1:]), a.dtype)
                 for a in out_avals]
        outs = sharded(*dev_inputs, *zeros)
        return [
            {name: np.asarray(outs[i]).reshape(NCORES, *out_avals[i].shape)[c]
             for i, name in enumerate(out_names)}
            for c in range(NCORES)
        ]

    _CACHE["run"] = run
    _CACHE["meta"] = (in_names, out_names, out_avals, sharded, mesh)
    return run


def kernel(**inputs) -> np.ndarray:
    run = get_runner()
    in_maps = prep_inputs(inputs)
    results = run(in_maps)
    return assemble_output(results).reshape(B, S, H).astype(np.float32)
